# revision 26
# baseline (speedup 1.0000x reference)
"""DGCNN (gnn_message_passing) Trainium2 Bass kernel, v2.

Strategy (data-parallel over graphs, 8 graphs per NeuronCore):
  - Host builds, per graph, the dense normalized propagation operator
    S[d, s] = (mult(s->d) + I) / deg_out[d]  (512x512 f32), shipped
    transposed as 4 chunks of [128, 512].
  - Layer-1 linear is folded on the host: LIN1 = (emb @ W1)[x] is shipped
    instead of h0 (flag FOLD_LIN1; layer-1 lin matmuls are skipped).
  - Graphs processed in pairs, graph-outer: each pair runs its 4 GCN
    layers on PE while the previous pair's sort-pooling tail runs on
    DVE/ACT/Pool, so the tail is hidden under PE time.
  - Per layer+graph on device: lin = h @ W as 4 chunk matmuls into one
    [128,512] PSUM bank, one copy to SBUF, then msgT = lin^T-chunks
    stationary x S^T chunks -> [128f, 512d] PSUM, tanh -> h (f32
    throughout: the top-64 sort is sensitive to ~1e-8 noise in h5, so
    the whole h chain must be bit-stable f32).
  - Sort-pooling tail per graph: exact stable ranks via comparison
    matrices (DVE/Pool alternating), one-hot rank matrix PT, top-64
    node indices extracted with a tiny iota matmul, converted to the
    wrapped int16 layout, and the pooled features gathered from h with
    gpsimd ap_gather (Pool engine) - no PE transposes or selection
    matmuls.
  - Conv1/maxpool/conv2/dense head batched across the 8 graphs.

Self-contained: hardcodes all shapes; no reads of /root/problem files.
"""

import sys

if "/opt/trn_rl_repo" not in sys.path:
    sys.path.insert(0, "/opt/trn_rl_repo")

import ml_dtypes
import numpy as np

import concourse.bacc as bacc
import concourse.mybir as mybir
import concourse.tile as tile
from concourse.bass_utils import run_bass_kernel_spmd

F32 = mybir.dt.float32
F32R = mybir.dt.float32r  # same bits/numerics as f32; 4x PE rate at >=256 cols
I16 = mybir.dt.int16
BF16 = mybir.dt.bfloat16

NUM_GRAPHS = 64
NPG = 512  # nodes per graph
N_TOTAL = NUM_GRAPHS * NPG
EMB = 128
DIMF = 128
NLAYERS = 4
K = 64
NCORES = 8
GPC = NUM_GRAPHS // NCORES  # graphs per core = 8
NLOC = GPC * NPG  # local nodes = 4096
LATENT = NLAYERS * DIMF + 1  # 513
DD = (K - 2) // 2 + 1  # 32
CONV2_LEN = DD - 5 + 1  # 28

HOT_W = 641     # wc 512 | w5 1 | idn(f32r) 128
COLD_W = 3406   # mj 2048 | ki 64 | iota 4 | w1a 64 | w1b 16 | w2 160 |
                # d1 896 | d2 2 | rm 128 | e1 16 | qm 4 | b1 b2 bd1 bd2 4

_NC_CACHE = {}


def _build(fold_lin1, debug):
    """Trace + compile the per-core Bass program (same on all 8 cores)."""
    nc = bacc.Bacc("TRN2", target_bir_lowering=False, debug=False,
                   num_devices=NCORES)

    # ---- per-core DRAM I/O ----
    # LIN1M[g] = (emb @ W1)[x] for graph g, chunk-major: [g][p][cc*128+f]
    #          = lin1[node cc*128+p of graph g, feat f]
    if fold_lin1:
        LIN1M = nc.dram_tensor("LIN1M", [GPC, 128, NPG], F32R,
                               kind="ExternalInput")
    else:
        H0T = nc.dram_tensor("H0T", [128, NLOC], F32, kind="ExternalInput")
    STD = nc.dram_tensor("STD", [GPC, 4, 128, NPG], F32R, kind="ExternalInput")
    # all small constants packed into two blobs (1 DMA each): HWDGE serial
    # overhead is ~630ns per DMA instruction, so DMA count dominates tiny
    # transfers. Layout documented in prepare_host.
    HOT = nc.dram_tensor("HOT", [128, HOT_W], F32R, kind="ExternalInput")
    COLD = nc.dram_tensor("COLD", [128, COLD_W], F32R, kind="ExternalInput")
    OUT = nc.dram_tensor("OUT", [2, GPC], F32, kind="ExternalOutput")
    if debug:
        DBG_H = nc.dram_tensor("DBG_H", [NLAYERS, 128, NLOC], F32,
                               kind="ExternalOutput")
        DBG_H5 = nc.dram_tensor("DBG_H5", [GPC, 1, NPG], F32,
                                kind="ExternalOutput")
        DBG_RANK = nc.dram_tensor("DBG_RANK", [GPC, 128, 4], F32,
                                  kind="ExternalOutput")
        DBG_IDX = nc.dram_tensor("DBG_IDX", [GPC, 128, 4], F32,
                                 kind="ExternalOutput")
        DBG_POOL = nc.dram_tensor("DBG_POOL", [NLAYERS, 128, GPC * K], F32,
                                  kind="ExternalOutput")
        DBG_POOL5 = nc.dram_tensor("DBG_POOL5", [1, GPC * K], F32,
                                   kind="ExternalOutput")
        DBG_Y2 = nc.dram_tensor("DBG_Y2", [32, GPC * CONV2_LEN], F32,
                                kind="ExternalOutput")

    TANH = mybir.ActivationFunctionType.Tanh
    RELU = mybir.ActivationFunctionType.Relu
    SIGM = mybir.ActivationFunctionType.Sigmoid
    ADD = mybir.AluOpType.add
    MULT = mybir.AluOpType.mult
    MAX = mybir.AluOpType.max
    IS_GT = mybir.AluOpType.is_gt
    IS_EQ = mybir.AluOpType.is_equal

    with tile.TileContext(nc) as tc:
        with (
            tc.tile_pool(name="const", bufs=1) as cp,
            tc.tile_pool(name="hg", bufs=6) as hp,          # [128,2048]/graph
            tc.tile_pool(name="st", bufs=32) as stp,        # [128,512]/chunk
            tc.tile_pool(name="lin", bufs=4) as linp,       # [128,512]
            tc.tile_pool(name="sc", bufs=3) as scp,         # compare scratch
            tc.tile_pool(name="vbp", bufs=3) as vbp,        # v broadcast
            tc.tile_pool(name="ptp", bufs=2) as ptp,        # ptt one-hots
            tc.tile_pool(name="sm", bufs=4) as smp,        # small tiles
            tc.tile_pool(name="idx", bufs=3) as idxp_pool,  # idx wrapped
            tc.tile_pool(name="xs", bufs=20) as xsp,        # endgame sel xts
            tc.tile_pool(name="ps512", bufs=2, space="PSUM") as ps5,
            tc.tile_pool(name="ps128", bufs=2, space="PSUM") as ps1,
            tc.tile_pool(name="psy1", bufs=1, space="PSUM") as psy,
        ):
            # ---- load order: pair-0 working set first ----
            lin1_sb = []

            def load_lin1(g):
                if fold_lin1:
                    t = linp.tile([128, NPG], F32R, tag="lin1", bufs=GPC,
                                  name=f"lin1_{g}")
                    nc.sync.dma_start(t[:], LIN1M[g, :, :])
                    return t
                return None

            st_sb = {}

            def load_st(g, chunked=False):
                # one DMA per graph (chunked for g=0 so the first prop can
                # start after the first quarter lands)
                t = stp.tile([128, 4 * NPG], F32R, tag="st", bufs=8,
                             name=f"st_t{g}")
                if chunked:
                    for c in range(4):
                        nc.sync.dma_start(t[:, c * NPG:(c + 1) * NPG],
                                          STD[g, c])
                else:
                    nc.sync.dma_start(t[:].rearrange("p (c w) -> p c w", c=4),
                                      STD[g].rearrange("c p w -> p c w"))
                st_sb[g] = [t[:, c * NPG:(c + 1) * NPG] for c in range(4)]

            if fold_lin1:
                lin1_sb = [load_lin1(0)]
                load_st(0, chunked=True)
                lin1_sb.append(load_lin1(1))
                load_st(1)
            else:
                h0 = hp.tile([128, NLOC], F32, tag="h0x", bufs=1)
                for s in range(0, 8):
                    nc.sync.dma_start(h0[:, s * 512:(s + 1) * 512],
                                      H0T[:, s * 512:(s + 1) * 512])
                load_st(0)
                load_st(1)
            hot = cp.tile([128, HOT_W], F32R, tag="hot")
            nc.sync.dma_start(hot[:], HOT[:])
            wc_sb = hot[:, 0:512]
            w5_sb = hot[:, 512:513]
            id_sb = hot[:, 513:641]
            if fold_lin1:
                lin1_sb.append(load_lin1(2))
            load_st(2)
            cold = cp.tile([128, COLD_W], F32R, tag="cold")
            nc.sync.dma_start(cold[:], COLD[:])
            mj_sb = cold[:, 0:2048].bitcast(F32)
            ki_sb = cold[:, 2048:2112].bitcast(F32)
            iota_sb = cold[:, 2112:2116]
            w1a_sb = cold[:, 2116:2180]
            w1b_sb = cold[0:1, 2180:2196]
            w2_sb = cold[0:16, 2196:2356]
            d1_sb = cold[0:32, 2356:3252]
            d2_sb = cold[0:32, 3252:3254]
            r_sb = cold[0:16, 3254:3382]
            e1_sb = cold[0:64, 3382:3398]
            qm_sb = cold[0:64, 3398:3402]
            b1_sb = cold[0:16, 3402:3403].bitcast(F32)
            b2_sb = cold[0:32, 3403:3404].bitcast(F32)
            bd1_sb = cold[0:32, 3404:3405].bitcast(F32)
            bd2_sb = cold[0:2, 3405:3406].bitcast(F32)
            for g in range(3, GPC):
                if fold_lin1:
                    lin1_sb.append(load_lin1(g))
                load_st(g)

            # per-graph state
            hgs = {}       # g -> [128, 4*512] tile (layers 1..4)
            vcols = {}     # g -> [128, 4] node-major h5
            vbs = {}       # g -> [128, 512] v broadcast
            ranks = {}     # g -> [128, 4]
            ptts = {}      # g -> [128, 4K] one-hot rank matrix
            idxw = {}      # g -> [128, 4] int16 wrapped indices
            # pooled features, all graphs side by side, one tile per layer
            pooled_sb = [cp.tile([128, GPC * K], F32R, tag=f"pool{l}",
                                 name=f"pool{l}")
                         for l in range(NLAYERS)]
            p5all = cp.tile([16, GPC * K], F32R, tag="p5all")
            y1p = psy.tile([16, GPC * K], F32, tag="y1p")
            y2all = cp.tile([32, GPC * CONV2_LEN], F32R, tag="y2all")
            y1 = cp.tile([16, GPC * K], F32, tag="y1")
            mp = cp.tile([16, GPC * K // 2], F32R, tag="mp")

            # ---------------- layer machinery ----------------
            def lin_stage(g, l):
                # returns SBUF [128, 4*128] chunk-major lin
                if l == 0 and fold_lin1:
                    return lin1_sb[g]
                lp = ps5.tile([128, NPG], F32, tag="linp", bufs=2)
                for cc in range(4):
                    if l == 0:
                        stat = h0[:, g * NPG + cc * 128:
                                  g * NPG + (cc + 1) * 128]
                    else:
                        stat = hgs[g][:, (l - 1) * NPG + cc * 128:
                                      (l - 1) * NPG + (cc + 1) * 128]
                    nc.tensor.matmul(
                        lp[:, cc * 128:(cc + 1) * 128], stat,
                        wc_sb[:, l * 128:(l + 1) * 128],
                        start=True, stop=True)
                ln = linp.tile([128, NPG], F32R, tag="lin")
                # ACT, not DVE: DVE runs multi-us rank-compare bursts and a
                # lin copy queued behind one stalls the next prop on PE
                nc.scalar.copy(ln[:], lp[:])
                return ln

            def prop_stage(g, l, ln):
                sp = ps5.tile([128, NPG], F32, tag="msgp", bufs=2)
                for cc in range(4):
                    nc.tensor.matmul(
                        sp[:], ln[:, cc * 128:(cc + 1) * 128],
                        st_sb[g][cc][:],
                        start=(cc == 0), stop=(cc == 3))
                nc.scalar.activation(
                    hgs[g][:, l * NPG:(l + 1) * NPG], sp[:], TANH)

            def alloc_h(g):
                hgs[g] = hp.tile([128, NLAYERS * NPG], F32R, tag="hg",
                                 name=f"h_{g}")

            def layers_pair(ga, gb):
                """All 4 GCN layers for graphs ga, gb, interleaved."""
                alloc_h(ga)
                alloc_h(gb)
                for l in range(NLAYERS):
                    lns = {g: lin_stage(g, l) for g in (ga, gb)}
                    for g in (ga, gb):
                        prop_stage(g, l, lns[g])
                    yield l

            # ---------------- tail stages ----------------
            lin5s = {}

            def sA(g):
                """layer-5 matvec (PE) + copy (DVE)."""
                hl = hgs[g]
                l5p = ps1.tile([128, 4], F32, tag="ps128")
                for cc in range(4):
                    # 1-col matmuls are invalid ISA in fp32r; use f32 views
                    nc.tensor.matmul(
                        l5p[:, cc:cc + 1],
                        hl[:, 3 * NPG + cc * 128:3 * NPG + (cc + 1) * 128]
                        .bitcast(F32),
                        w5_sb[:].bitcast(F32), start=True, stop=True)
                lin5 = smp.tile([128, 4], F32R, tag="lin5")
                nc.vector.tensor_copy(lin5[:], l5p[:])
                lin5s[g] = lin5

            def sB(g):
                """msg5 = S @ lin5 (PE, 16 tiny) + tanh (ACT)."""
                lin5 = lin5s[g]
                m5p = ps1.tile([128, 4], F32, tag="ps128")
                for dc in range(4):
                    for sc in range(4):
                        nc.tensor.matmul(
                            m5p[:, dc:dc + 1],
                            st_sb[g][sc][:, dc * 128:(dc + 1) * 128]
                            .bitcast(F32),
                            lin5[:, sc:sc + 1].bitcast(F32),
                            start=(sc == 0), stop=(sc == 3))
                vcol = smp.tile([128, 4], F32R, tag="vcol")
                nc.scalar.activation(vcol[:], m5p[:], TANH)
                vcols[g] = vcol

            def sC(g):
                """h5 row form + broadcast. Four column transposes land the
                row on partition 0 directly (no DMA: a DMA costs ~2.3us of
                semaphore latency on the tail chain; partition_broadcast
                requires a partition-0 source)."""
                vcol = vcols[g]
                vtp = ps1.tile([1, NPG], F32R, tag="ps128")
                for cc in range(4):
                    nc.tensor.matmul(
                        vtp[0:1, cc * 128:(cc + 1) * 128],
                        vcol[:, cc:cc + 1], id_sb[:],
                        start=(cc == 0), stop=(cc == 3), is_transpose=True)
                h5row = smp.tile([1, NPG], F32R, tag="h5r", bufs=3)
                nc.vector.tensor_copy(h5row[:], vtp[:])
                vb = vbp.tile([128, NPG], F32R, tag="vb")
                nc.gpsimd.partition_broadcast(vb[:], h5row[0:1, :])
                vbs[g] = vb
                if debug:
                    nc.sync.dma_start(DBG_H5[g, :, :], h5row[:].bitcast(F32))

            def _cmp_eng(g):
                # DVE only: TensorScalar/ScalarTensorTensor are not valid
                # Pool-engine opcodes for the real walrus codegen
                return nc.vector

            def sD(g):
                """exact stable ranks (engine by graph parity)."""
                vb, vcol = vbs[g], vcols[g]
                eng = _cmp_eng(g)
                rank = smp.tile([128, 4], F32, tag="rank")
                for cc in range(4):
                    t1 = scp.tile([128, NPG], F32, tag="tt")
                    ra = smp.tile([128, 2], F32, tag="ra")
                    eng.tensor_scalar(
                        out=t1[:], in0=vb[:],
                        scalar1=vcol[:, cc:cc + 1].bitcast(F32),
                        scalar2=None, op0=IS_GT, op1=ADD,
                        accum_out=ra[:, 0:1])
                    t2 = scp.tile([128, NPG], F32, tag="tt")
                    eng.scalar_tensor_tensor(
                        out=t2[:], in0=vb[:],
                        scalar=vcol[:, cc:cc + 1].bitcast(F32),
                        in1=mj_sb[:, cc * NPG:(cc + 1) * NPG],
                        op0=IS_EQ, op1=MULT, accum_out=ra[:, 1:2])
                    eng.tensor_tensor(
                        out=rank[:, cc:cc + 1], in0=ra[:, 0:1],
                        in1=ra[:, 1:2], op=ADD)
                ranks[g] = rank
                if debug:
                    nc.sync.dma_start(DBG_RANK[g, :, :], rank[:])

            def sE(g):
                """one-hot rank matrix (columns in wrapped-permuted order)."""
                rank = ranks[g]
                eng = _cmp_eng(g)
                ptt = ptp.tile([128, 4 * K], F32R, tag="pt")
                for cc in range(4):
                    eng.tensor_scalar(
                        out=ptt[:, cc * K:(cc + 1) * K], in0=ki_sb[:],
                        scalar1=rank[:, cc:cc + 1], scalar2=None, op0=IS_EQ)
                ptts[g] = ptt

            def sF(g):
                """ordered top-64 node indices, int16 wrapped for ap_gather."""
                ptt = ptts[g]
                # col64[q] = index of the node with rank perm(q); KI's
                # permutation makes the downstream folds land each index at
                # iw[p, s] = idx[s*16+p], the ap_gather wrapped layout.
                cxp = ps1.tile([K, 1], F32, tag="ps128")
                for cc in range(4):
                    nc.tensor.matmul(cxp[:],
                                     ptt[:, cc * K:(cc + 1) * K].bitcast(F32),
                                     iota_sb[:, cc:cc + 1].bitcast(F32),
                                     start=(cc == 0), stop=(cc == 3))
                c64 = smp.tile([K, 1], F32, tag="c64")
                nc.vector.tensor_copy(c64[:], cxp[:])
                m64 = smp.tile([K, 4], F32R, tag="m64")
                nc.vector.tensor_scalar(out=m64[:], in0=qm_sb[:],
                                        scalar1=c64[:, 0:1], scalar2=None,
                                        op0=MULT)
                wqp = ps1.tile([16, 4], F32, tag="ps128")
                nc.tensor.matmul(wqp[:], e1_sb[:], m64[:],
                                 start=True, stop=True)
                wq = smp.tile([16, 4], F32R, tag="wq")
                nc.vector.tensor_copy(wq[:], wqp[:])
                wfp = ps1.tile([128, 4], F32, tag="ps128")
                nc.tensor.matmul(wfp[:], r_sb[:], wq[:],
                                 start=True, stop=True)
                iw = idxp_pool.tile([128, 4], I16, tag="iw")
                nc.vector.tensor_copy(iw[:], wfp[:])
                idxw[g] = iw
                if debug:
                    dbgi = smp.tile([128, 4], F32, tag="dbgi")
                    nc.vector.tensor_copy(dbgi[:], iw[:])
                    nc.sync.dma_start(DBG_IDX[g, :, :], dbgi[:])

            def sG(g, pool5_on_pe=False):
                """gather pooled features on the Pool engine."""
                iw = idxw[g]
                hl = hgs[g]
                for l in range(NLAYERS):
                    nc.gpsimd.ap_gather(
                        pooled_sb[l][:, g * K:(g + 1) * K],
                        hl[:, l * NPG:(l + 1) * NPG], iw[:],
                        channels=128, num_elems=NPG, d=1, num_idxs=K)
                if pool5_on_pe:
                    # v[idx] via selection matmul; undo the column
                    # permutation with a strided copy out of PSUM.
                    vcol, ptt = vcols[g], ptts[g]
                    p5p = ps1.tile([1, K], F32, tag="ps128")
                    for cc in range(4):
                        nc.tensor.matmul(p5p[:],
                                         vcol[:, cc:cc + 1],
                                         ptt[:, cc * K:(cc + 1) * K],
                                         start=(cc == 0), stop=(cc == 3))
                    dstv = p5all[0:1, g * K:(g + 1) * K].rearrange(
                        "a (s p) -> a p s", p=16)
                    srcv = p5p[0:1, :].rearrange("a (p s) -> a p s", s=4)
                    nc.vector.tensor_copy(dstv, srcv)
                else:
                    nc.gpsimd.ap_gather(
                        p5all[:, g * K:(g + 1) * K], vbs[g][0:16, :],
                        iw[0:16, :], channels=16, num_elems=NPG, d=1,
                        num_idxs=K)

            def sHa(g):
                """conv1 + relu + maxpool for graph g."""
                for l in range(NLAYERS):
                    nc.tensor.matmul(y1p[:, g * K:(g + 1) * K],
                                     w1a_sb[:, l * 16:(l + 1) * 16],
                                     pooled_sb[l][:, g * K:(g + 1) * K],
                                     start=(l == 0), stop=False)
                nc.tensor.matmul(y1p[:, g * K:(g + 1) * K], w1b_sb[:],
                                 p5all[0:1, g * K:(g + 1) * K],
                                 start=False, stop=True)
                nc.scalar.activation(y1[:, g * K:(g + 1) * K],
                                     y1p[:, g * K:(g + 1) * K], RELU,
                                     bias=b1_sb[:, 0:1])
                y1v = y1[:, g * K:(g + 1) * K].rearrange(
                    "p (a b) -> p a b", b=2)
                nc.vector.tensor_tensor(
                    out=mp[:, g * DD:(g + 1) * DD], in0=y1v[:, :, 0:1],
                    in1=y1v[:, :, 1:2], op=MAX)

            def sHb(g):
                """conv2 + relu for graph g."""
                y2p = ps1.tile([32, CONV2_LEN], F32, tag="ps128")
                for t5 in range(5):
                    nc.tensor.matmul(
                        y2p[:],
                        w2_sb[:, t5 * 32:(t5 + 1) * 32],
                        mp[:, g * DD + t5:g * DD + t5 + CONV2_LEN],
                        start=(t5 == 0), stop=(t5 == 4))
                nc.scalar.activation(
                    y2all[:, g * CONV2_LEN:(g + 1) * CONV2_LEN], y2p[:],
                    RELU, bias=b2_sb[:, 0:1])

            def sH(g):
                """per-graph conv1 + relu + maxpool + conv2 + relu."""
                for l in range(NLAYERS):
                    nc.tensor.matmul(y1p[:, g * K:(g + 1) * K],
                                     w1a_sb[:, l * 16:(l + 1) * 16],
                                     pooled_sb[l][:, g * K:(g + 1) * K],
                                     start=(l == 0), stop=False)
                nc.tensor.matmul(y1p[:, g * K:(g + 1) * K], w1b_sb[:],
                                 p5all[0:1, g * K:(g + 1) * K],
                                 start=False, stop=True)
                nc.scalar.activation(y1[:, g * K:(g + 1) * K],
                                     y1p[:, g * K:(g + 1) * K], RELU,
                                     bias=b1_sb[:, 0:1])
                y1v = y1[:, g * K:(g + 1) * K].rearrange(
                    "p (a b) -> p a b", b=2)
                nc.vector.tensor_tensor(
                    out=mp[:, g * DD:(g + 1) * DD], in0=y1v[:, :, 0:1],
                    in1=y1v[:, :, 1:2], op=MAX)
                y2p = ps1.tile([32, CONV2_LEN], F32, tag="ps128")
                for t5 in range(5):
                    nc.tensor.matmul(
                        y2p[:],
                        w2_sb[:, t5 * 32:(t5 + 1) * 32],
                        mp[:, g * DD + t5:g * DD + t5 + CONV2_LEN],
                        start=(t5 == 0), stop=(t5 == 4))
                nc.scalar.activation(
                    y2all[:, g * CONV2_LEN:(g + 1) * CONV2_LEN], y2p[:],
                    RELU, bias=b2_sb[:, 0:1])


            # ---------------- schedule ----------------
            # Groups: 3 pairs then two singles (graphs 6, 7). Singles let
            # graph 6's DVE rank burst overlap graph 7's layers, so only
            # one compare chain is terminal. Tail stages are placed on a
            # global slot grid (one slot per layer issued): sA at the
            # graph's own last layer slot e, then B@e+1, C@e+2, D,E@e+3+i,
            # F@e+4+i, G@e+5+i, H@e+6+i (i = index within group) so the
            # per-graph 3.7us DVE compare bursts never pile up in one slot.
            from collections import defaultdict
            groups = [(0, 1), (2, 3), (4, 5), (6,), (7,)]
            slots = defaultdict(list)
            start = 0
            for grp in groups:
                e = start + NLAYERS - 1
                for i, g in enumerate(grp):
                    slots[e].append((sA, g))
                    slots[e + 1].append((sB, g))
                    slots[e + 2].append((sC, g))
                    slots[e + 3 + i].append((sD, g))
                    slots[e + 3 + i].append((sE, g))
                    slots[e + 4 + i].append((sF, g))
                    slots[e + 5 + i].append((sG, g))
                    slots[e + 6 + i].append((sH, g))
                start += NLAYERS
            slot = 0
            for grp in groups:
                for g in grp:
                    alloc_h(g)
                for l in range(NLAYERS):
                    lns = {g: lin_stage(g, l) for g in grp}
                    for g in grp:
                        prop_stage(g, l, lns[g])
                    for fn, g in slots.pop(slot, ()):
                        fn(g)
                    slot += 1
            # epilogue: remaining tail stages in slot order
            for s in sorted(slots):
                for fn, g in slots[s]:
                    fn(g)

            if debug:
                for l in range(NLAYERS):
                    for g in range(GPC):
                        nc.sync.dma_start(
                            DBG_H[l, :, g * NPG:(g + 1) * NPG],
                            hgs[g][:, l * NPG:(l + 1) * NPG].bitcast(F32))
                    nc.sync.dma_start(DBG_POOL[l, :, :], pooled_sb[l][:].bitcast(F32))
                nc.sync.dma_start(DBG_POOL5[:], p5all[0:1, :].bitcast(F32))

            # ---------------- final head (y2all filled per-graph) ----------
            if debug:
                nc.sync.dma_start(DBG_Y2[:], y2all[:].bitcast(F32))

            h1p = ps1.tile([32, GPC], F32, tag="ps128")
            y2v = y2all[:].rearrange("p (g t) -> p g t", t=CONV2_LEN)
            for t5 in range(CONV2_LEN):
                nc.tensor.matmul(h1p[:], d1_sb[:, t5 * 32:(t5 + 1) * 32],
                                 y2v[:, :, t5:t5 + 1],
                                 start=(t5 == 0), stop=(t5 == CONV2_LEN - 1))
            h1s = smp.tile([32, GPC], F32R, tag="h1s", bufs=1)
            nc.scalar.activation(h1s[:], h1p[:], RELU, bias=bd1_sb[:, 0:1])
            dfp = ps1.tile([2, GPC], F32, tag="ps128")
            nc.tensor.matmul(dfp[:], d2_sb[:], h1s[:], start=True, stop=True)
            pr = smp.tile([2, GPC], F32, tag="pr", bufs=1)
            nc.scalar.activation(pr[:], dfp[:], SIGM, bias=bd2_sb[:, 0:1])
            nc.sync.dma_start(OUT[:], pr[:])

    nc.compile()
    return nc


def _get_nc(fold_lin1, debug):
    key = (fold_lin1, debug)
    if key not in _NC_CACHE:
        _NC_CACHE[key] = _build(fold_lin1, debug)
    return _NC_CACHE[key]


def prepare_host(inputs, fold_lin1=True):
    """All host-side index preprocessing + per-core input maps."""
    x = np.asarray(inputs["x"]).astype(np.int64)
    edge_index = np.asarray(inputs["edge_index"]).astype(np.int64)
    emb = np.ascontiguousarray(np.asarray(inputs["emb"], dtype=np.float32))
    W_convs = np.asarray(inputs["W_convs"], dtype=np.float32)
    conv1_w = np.asarray(inputs["conv1_w"], dtype=np.float32)
    conv1_b = np.asarray(inputs["conv1_b"], dtype=np.float32)
    conv2_w = np.asarray(inputs["conv2_w"], dtype=np.float32)
    conv2_b = np.asarray(inputs["conv2_b"], dtype=np.float32)
    d1_w = np.asarray(inputs["d1_w"], dtype=np.float32)
    d1_b = np.asarray(inputs["d1_b"], dtype=np.float32)
    d2_w = np.asarray(inputs["d2_w"], dtype=np.float32)
    d2_b = np.asarray(inputs["d2_b"], dtype=np.float32)
    W_last = np.asarray(inputs["W_last"], dtype=np.float32)

    src, dst = edge_index[0], edge_index[1]
    deg = (np.bincount(src, minlength=N_TOTAL) + 1).astype(np.float32)
    invdeg = (np.float32(1.0) / deg).astype(np.float32)
    gid = dst >> 9
    flat = (gid * NPG + (dst & 511)) * NPG + (src & 511)
    A = np.bincount(flat, minlength=NUM_GRAPHS * NPG * NPG)
    A = A.astype(np.float32).reshape(NUM_GRAPHS, NPG, NPG)
    idx = np.arange(NPG)
    A[:, idx, idx] += 1.0
    S = A * invdeg.reshape(NUM_GRAPHS, NPG, 1)
    ST = np.ascontiguousarray(S.transpose(0, 2, 1)).reshape(
        NUM_GRAPHS, 4, 128, NPG)

    w1 = np.ascontiguousarray(conv1_w[:, 0, :].T)  # [513, 16]

    # ---- HOT blob [128, 577]: wc 0:512 | w5 512 | idn(bf16) 513:577 ----
    hot = np.zeros((128, HOT_W), np.float32)
    hot[:, 0:512] = W_convs.transpose(1, 0, 2).reshape(128, 512)
    hot[:, 512:513] = W_last
    hot[:, 513:641] = np.eye(128, dtype=np.float32)

    # ---- COLD blob [128, 3406] ----
    cold = np.zeros((128, COLD_W), np.float32)
    cold[:, 0:2048] = (np.arange(NPG)[None, None, :]
                       < (np.arange(4)[:, None, None] * 128
                          + np.arange(128)[None, :, None])
                       ).astype(np.float32).transpose(1, 0, 2).reshape(128, 2048)
    cold[:, 2048:2112] = np.broadcast_to(
        ((np.arange(K) % 4) * 16 + np.arange(K) // 4)
        .astype(np.float32), (128, K))
    cold[:, 2112:2116] = (np.arange(4)[None, :] * 128
                          + np.arange(128)[:, None]).astype(np.float32)
    cold[:, 2116:2180] = w1[:512].reshape(4, 128, 16).transpose(1, 0, 2) \
        .reshape(128, 64)
    cold[0:1, 2180:2196] = w1[512:513]
    cold[0:16, 2196:2356] = conv2_w.transpose(2, 1, 0) \
        .transpose(1, 0, 2).reshape(16, 160)
    cold[0:32, 2356:3252] = d1_w.reshape(DD, CONV2_LEN * 32)
    cold[0:32, 3252:3254] = (d2_w.astype(np.float64)
                             @ np.array([[1.0, -1.0], [-1.0, 1.0]])
                             ).astype(np.float32)
    cold[0:16, 3254:3382] = (np.arange(128)[None, :] % 16
                             == np.arange(16)[:, None]).astype(np.float32)
    cold[0:64, 3382:3398] = (np.arange(64)[:, None] // 4
                             == np.arange(16)[None, :]).astype(np.float32)
    cold[0:64, 3398:3402] = (np.arange(64)[:, None] % 4
                             == np.arange(4)[None, :]).astype(np.float32)
    cold[0:16, 3402:3403] = conv1_b.reshape(16, 1)
    cold[0:32, 3403:3404] = conv2_b.reshape(32, 1)
    cold[0:32, 3404:3405] = d1_b.reshape(32, 1)
    cold[0:2, 3405:3406] = (np.array([[1.0, -1.0], [-1.0, 1.0]])
                            @ d2_b.reshape(2, 1)).astype(np.float32)

    shared = {"HOT": hot, "COLD": cold}

    if fold_lin1:
        lin1 = emb @ W_convs[0]        # [1000, 128] f32
        h0lin = lin1[x]                # [N, 128]
    h0 = emb[x]

    in_maps = []
    for c in range(NCORES):
        m = dict(shared)
        if fold_lin1:
            loc = h0lin[c * NLOC:(c + 1) * NLOC]  # [4096, 128]
            # [g][p][cc*128+f] = lin1[g*512+cc*128+p, f]
            lm = loc.reshape(GPC, 4, 128, 128).transpose(0, 2, 1, 3)
            m["LIN1M"] = np.ascontiguousarray(
                lm.reshape(GPC, 128, NPG))
        else:
            m["H0T"] = np.ascontiguousarray(h0[c * NLOC:(c + 1) * NLOC].T)
        m["STD"] = np.ascontiguousarray(ST[c * GPC:(c + 1) * GPC])
        in_maps.append(m)
    return in_maps


def run(inputs, fold_lin1=True, debug=False, **spmd_kwargs):
    in_maps = prepare_host(inputs, fold_lin1)
    nc = _get_nc(fold_lin1, debug)
    res = run_bass_kernel_spmd(nc, in_maps, core_ids=list(range(NCORES)),
                               **spmd_kwargs)
    out = np.empty((NUM_GRAPHS, 2), dtype=np.float32)
    for c in range(NCORES):
        out[c * GPC:(c + 1) * GPC, :] = res.results[c]["OUT"].T
    return out, res


def kernel(**inputs):
    out, _ = run(inputs, fold_lin1=True)
    return out

